# revision 7
# baseline (speedup 1.0000x reference)
"""BitLinear (packed +/-1 linear layer) Trainium2 kernel.

Math: out[b,o] = sum_k a[b,k]*w[o,k] + bias[o], where a/w are +/-1 values
bit-packed LSB-first into bytes (stored as int32 0..255).

Device strategy (8 NeuronCores, data-parallel over batch):
  - Each core gets B/8 = 1024 batch rows; the full weight matrix is
    replicated.
  - Unpack trick: one DVE tensor_scalar per (staged tile, bit) moves bit i
    of every byte to bit position 6 and masks: y = (x << (6-i)) & 0x4040 on
    uint16 views (i=7 uses >> 1). Byte 0x40 bitcast to fp8e4 reads as 2.0,
    so unpacked operands take values {0, 2.0} with no cast op, and the
    16-bit dtype enables the DVE 4x perf mode.
  - fp8 DoubleRowSwInterleave matmuls (256-deep contraction per
    instruction). HW semantics (verified on HW + bass_interp source): the
    moving operand keeps the DoubleRow plane layout [Ki, 2, N]; the
    stationary operand is read CONTIGUOUSLY as interleaved pairs in
    reversed column order [A127,B127,...,A0,B0]. The contiguous weight read
    makes LDWEIGHTS ~4x cheaper than HW-interleave DoubleRow (measured
    ~134.6 vs ~166.4 ns marginal per N=512 matmul at stationary reuse 4).
  - k-pairing: contraction pair = (bit i of byte-row 2P*128+p, bit i of
    byte-row (2P+1)*128+p); group g = i + 8P. Host stages activations
    pre-interleaved along the pair dim and batch-reversed within each
    128-batch tile, so the same one-op-per-bit u16 unpack directly emits
    the SwInterleave stationary layout. Weight staging is unchanged.
  - Redundant InstLdweights (same stationary AP) are deduped post-schedule:
    the PE keeps the stationary operand loaded across matmuls (verified
    exact on HW), so each a2 tile is loaded once per NOQ matmuls.
  - Identity: with a = 2*alpha-1, w = 2*omega-1 (alpha,omega in {0,1}):
      out = 4*M - 2*rowsum(alpha) - 2*rowsum(omega) + K + bias
          = psum + r2[b] + c[o]
    where r2[b] = -2*popcount_rows(A), c[o] = bias + K - 2*popcount_rows(W)
    are cheap linear-time host precomputes (fp32-exact integers). The
    epilogue is one scalar_tensor_tensor per psum bank:
      out = (psum + r2_per_partition) + c_tile.

Everything is exact: products in {0,4}, fp32 PSUM accumulation of integers
<= 2^14, so the only rounding differences vs the fp32 reference are in the
final bias add (~1 ulp).
"""

import os
import sys

import numpy as np

for _p in ("/opt/trn_rl_repo", "/root/.axon_site/_ro/trn_rl_repo"):
    if os.path.isdir(_p) and _p not in sys.path:
        sys.path.append(_p)

BATCH = 8192
IN_FEATURES = 4096
OUT_FEATURES = 4096
PACKED_LEN = IN_FEATURES // 8  # 512
N_CORES = 8
P = 128

_NC_CACHE: dict = {}
LAST_RESULTS = None  # stash of the most recent BassKernelResults (for test.py)


def build_program(B, O, K, n_devices=N_CORES, o_half=2048, reps=1,
                  mm_reps=1, up_reps=1, out_bufs=3, stage_bufs=4,
                  psum_bufs=2, mm_inner=False, dedupe_ldw=True,
                  pairwise_unpack=True, a_engine="vector"):
    """Emit the per-core Bass/Tile program. SPMD: same program every core.

    reps>1 repeats the whole compute body (identical writes) so test.py can
    measure pure device time as (T(reps=R) - T(reps=1)) / (R - 1).
    mm_reps / up_reps repeat only the matmul block / only the unpack ops --
    engine-rate microbenchmarks via the same delta method. mm_inner=True
    repeats each stationary group consecutively (rate probe only; numerics
    are garbage for mm_reps>1)."""
    import concourse.bass as bass  # noqa: F401
    import concourse.mybir as mybir
    import concourse.tile as tile
    from concourse import bacc

    KP = K // 8  # packed k rows (512)
    NT = KP // P  # staged byte-row tiles (4)
    NK2 = K // 256  # contraction groups (16): g = bit + 8*row_pair
    OH = min(O, o_half)  # o columns processed per outer phase
    NH = O // OH
    NOQ = OH // 512  # psum banks per phase
    NB = B // P  # batch tiles
    assert KP % P == 0 and O % OH == 0 and OH % 512 == 0 and B % P == 0
    assert NT == 4 and NK2 == 16

    u8 = mybir.dt.uint8
    u16 = mybir.dt.uint16
    f32 = mybir.dt.float32
    fp8 = mybir.dt.float8e4
    shl = mybir.AluOpType.logical_shift_left
    shr = mybir.AluOpType.logical_shift_right
    band = mybir.AluOpType.bitwise_and
    add = mybir.AluOpType.add
    SWI = mybir.MatmulPerfMode.DoubleRowSwInterleave

    nc = bacc.Bacc(
        "TRN2",
        target_bir_lowering=False,
        debug=False,
        num_devices=n_devices,
    )

    # at: [128, 4B] = two halves (row-pair P), each [128, 2B] interleaved
    # pairs, batch-reversed per 128-block. wt: [KP, O] rows-on-partitions.
    at_d = nc.dram_tensor("at", [P, 4 * B], u8, kind="ExternalInput").ap()
    wt_d = nc.dram_tensor("wt", [KP, O], u8, kind="ExternalInput").ap()
    c_d = nc.dram_tensor("c_rep", [P, O], f32, kind="ExternalInput").ap()
    r2_d = nc.dram_tensor("r2t", [P, NB], f32, kind="ExternalInput").ap()
    out_d = nc.dram_tensor("out", [B, O], f32, kind="ExternalOutput").ap()

    def unpack_ops(i):
        # Move bit i of every byte to bit 6 and mask; uint16 views keep both
        # byte lanes independent for shifts <= 6 left / 1 right.
        return (shr, 1, band, 0x4040) if i == 7 else (shl, 6 - i, band, 0x4040)

    with tile.TileContext(nc) as tc:
        with (
            tc.tile_pool(name="consts", bufs=1) as cpool,
            tc.tile_pool(name="a2", bufs=1) as a2pool,
            tc.tile_pool(name="w2", bufs=1) as w2pool,
            tc.tile_pool(name="stage", bufs=stage_bufs) as spool,
            tc.tile_pool(name="outs", bufs=out_bufs) as opool,
            tc.tile_pool(name="psum", bufs=psum_bufs, space="PSUM") as ppool,
        ):
            c_rep = cpool.tile([P, O], f32, name="c_rep_t")
            r2t = cpool.tile([P, NB], f32, name="r2t_t")
            consts_loaded = False

            # repeat body for delta timing (rep>0 re-does identical work)
            for rep in range(reps):
              # ---- unpack activations (whole batch shard, kept resident):
              # a2[g] holds the SwInterleave stationary layout directly ----
              a2 = [
                  a2pool.tile([P, 2 * B], u8, name=f"a2_{g}")
                  for g in range(NK2)
              ]
              a_eng = getattr(nc, a_engine)
              for h in range(NH):
                  # ---- unpack this phase's weight slice; ops are emitted
                  # pairwise across the two staged row-block tiles so
                  # contraction group g is complete after 2(g%8)+2 ops and
                  # the PE's sequential accumulation is fed in order ----
                  w2 = [
                      w2pool.tile([P, 2, OH], u8, name=f"w2_{g}")
                      for g in range(NK2)
                  ]
                  for tp in range(NT // 2):
                      wt_sts = []
                      for e in range(2):
                          t = 2 * tp + e
                          wt_st = spool.tile([P, OH], u8, name="wt_st")
                          nc.sync.dma_start(
                              out=wt_st,
                              in_=wt_d[t * P : (t + 1) * P, h * OH : (h + 1) * OH],
                          )
                          wt_sts.append(wt_st)
                      if h == 0:
                          at_st = spool.tile([P, 2 * B], u8, name="at_st")
                          nc.sync.dma_start(
                              out=at_st,
                              in_=at_d[:, tp * 2 * B : (tp + 1) * 2 * B],
                          )
                      if not consts_loaded:
                          # emitted after the first stage DMAs: the epilogue
                          # constants (2 MiB) must not serialize the DMA
                          # queue ahead of the PE-critical first tiles
                          consts_loaded = True
                          nc.sync.dma_start(out=c_rep, in_=c_d)
                          nc.sync.dma_start(out=r2t, in_=r2_d)
                      for _ur in range(up_reps):
                        for i in range(8):
                          op0, s1, op1, s2 = unpack_ops(i)
                          g = i + 8 * tp
                          order = (
                              [("w", 0), ("w", 1), ("a", 0)]
                              if pairwise_unpack
                              else [("a", 0), ("w", 0), ("w", 1)]
                          )
                          for kind, e in order:
                              if kind == "a":
                                  if h == 0:
                                      a_eng.tensor_scalar(
                                          out=a2[g].bitcast(u16),
                                          in0=at_st.bitcast(u16),
                                          scalar1=s1,
                                          scalar2=s2,
                                          op0=op0,
                                          op1=op1,
                                      )
                              else:
                                  nc.vector.tensor_scalar(
                                      out=w2[g][:, e, :].bitcast(u16),
                                      in0=wt_sts[e].bitcast(u16),
                                      scalar1=s1,
                                      scalar2=s2,
                                      op0=op0,
                                      op1=op1,
                                  )

                  # ---- matmul + epilogue ----
                  for b in range(NB):
                      out_st = opool.tile([P, OH], f32, name="out_st")
                      psums = [
                          ppool.tile([P, 512], f32, name=f"ps_{oq}")
                          for oq in range(NOQ)
                      ]
                      if mm_inner:
                        for g in range(NK2):
                          lhsT = a2[g][:, b * 256 : (b + 1) * 256].bitcast(fp8)
                          for _mr in range(mm_reps):
                            for oq in range(NOQ):
                              nc.tensor.matmul(
                                  psums[oq],
                                  lhsT,
                                  w2[g][:, :, oq * 512 : (oq + 1) * 512].bitcast(fp8),
                                  start=(g == 0),
                                  stop=(g == NK2 - 1),
                                  perf_mode=SWI,
                              )
                      else:
                        for _mr in range(mm_reps):
                          for g in range(NK2):
                            lhsT = a2[g][:, b * 256 : (b + 1) * 256].bitcast(fp8)
                            for oq in range(NOQ):
                              nc.tensor.matmul(
                                  psums[oq],
                                  lhsT,
                                  w2[g][:, :, oq * 512 : (oq + 1) * 512].bitcast(fp8),
                                  start=(g == 0),
                                  stop=(g == NK2 - 1),
                                  perf_mode=SWI,
                              )
                      for oq in range(NOQ):
                          osl = slice(oq * 512, (oq + 1) * 512)
                          csl = slice(h * OH + oq * 512, h * OH + (oq + 1) * 512)
                          nc.vector.scalar_tensor_tensor(
                              out=out_st[:, osl],
                              in0=psums[oq],
                              scalar=r2t[:, b : b + 1],
                              in1=c_rep[:, csl],
                              op0=add,
                              op1=add,
                          )
                          # store each bank as soon as its epilogue lands so
                          # the final DMAs overlap the remaining epilogues
                          nc.sync.dma_start(
                              out=out_d[b * P : (b + 1) * P, csl],
                              in_=out_st[:, osl],
                          )

    if dedupe_ldw:
        _dedupe_ldweights(nc, mybir)
    nc.compile()
    return nc


def _dedupe_ldweights(nc, mybir):
    """Drop InstLdweights whose weights AP is identical to the previous one
    (the PE keeps the stationary operand loaded; verified exact on HW). Any
    sync waits/updates on a dropped load move to the following matmul."""
    for f in nc.m.functions:
        for blk in f.blocks:
            last_sig = None
            keep = []
            pend_w = []
            pend_u = []
            for inst in blk.instructions:
                tn = type(inst).__name__
                if tn == "InstLdweights":
                    sig = str(inst.ins[0])
                    if sig == last_sig:
                        si = inst.sync_info
                        if si is not None:
                            pend_w.extend(si.on_wait)
                            pend_u.extend(si.on_update)
                        continue
                    last_sig = sig
                elif tn == "InstMatmult":
                    if pend_w or pend_u:
                        si = inst.sync_info
                        if si is None:
                            inst.sync_info = mybir.SyncInfo(
                                on_wait=list(pend_w), on_update=list(pend_u)
                            )
                        else:
                            si.on_wait = list(si.on_wait) + pend_w
                            si.on_update = list(si.on_update) + pend_u
                        pend_w = []
                        pend_u = []
                    if getattr(inst, "is_transpose", False):
                        last_sig = None
                keep.append(inst)
            assert not pend_w and not pend_u
            blk.instructions = keep


_POP = np.unpackbits(np.arange(256, dtype=np.uint8)[:, None], axis=1).sum(1)


def _prep_inputs(input_packed, weight_packed, bias, B, O, K, n_cores):
    """Host-side linear-time preprocessing: cast/transpose/shard + popcount
    rank-1 correction terms + SwInterleave activation layout."""
    bsh = B // n_cores
    NB = bsh // P
    A8 = input_packed.astype(np.uint8)  # [B, KP]
    W8 = weight_packed.astype(np.uint8)  # [O, KP]
    rA = _POP[A8].sum(1, dtype=np.int64)  # [B]
    rW = _POP[W8].sum(1, dtype=np.int64)  # [O]
    c = (bias.astype(np.float64) + K - 2.0 * rW).astype(np.float32)
    c_rep = np.ascontiguousarray(np.broadcast_to(c, (P, O)))
    r2 = (-2.0 * rA).astype(np.float32)
    wt = np.ascontiguousarray(W8.T)  # [KP, O]

    # SwInterleave activation staging: R[q, p, m] = byte-row q*128+p.
    # Half hp pairs rows (q=2hp, q=2hp+1): positions [p, 2c+e] with
    # c = batch reversed within each 128-block.
    at_all = A8.T.reshape(4, P, B)  # [q, p, B]
    halves = []
    for hp in range(2):
        st = np.stack([at_all[2 * hp], at_all[2 * hp + 1]], axis=-1)  # [P,B,2]
        st = st.reshape(P, B // P, P, 2)[:, :, ::-1, :]  # reverse per block
        halves.append(st.reshape(P, 2 * B))
    at_swi = np.concatenate(halves, axis=1)  # [P, 4B]

    in_maps = []
    for ci in range(n_cores):
        sl = slice(ci * bsh, (ci + 1) * bsh)
        # per-core slice: batch columns of each half
        at_core = np.concatenate(
            [
                at_swi[:, 2 * ci * bsh : 2 * (ci + 1) * bsh],
                at_swi[:, 2 * B + 2 * ci * bsh : 2 * B + 2 * (ci + 1) * bsh],
            ],
            axis=1,
        )
        in_maps.append(
            {
                "at": np.ascontiguousarray(at_core),
                "wt": wt,
                "c_rep": c_rep,
                "r2t": np.ascontiguousarray(r2[sl].reshape(NB, P).T),
            }
        )
    return in_maps


def kernel(input_packed, weight_packed, bias):
    global LAST_RESULTS
    from concourse.bass_utils import run_bass_kernel_spmd

    input_packed = np.asarray(input_packed)
    weight_packed = np.asarray(weight_packed)
    bias = np.asarray(bias)
    B, KP = input_packed.shape
    O = weight_packed.shape[0]
    K = KP * 8
    key = (B, O, K, N_CORES)
    if key not in _NC_CACHE:
        _NC_CACHE[key] = build_program(B // N_CORES, O, K, n_devices=N_CORES)
    nc = _NC_CACHE[key]

    in_maps = _prep_inputs(input_packed, weight_packed, bias, B, O, K, N_CORES)
    res = run_bass_kernel_spmd(nc, in_maps, list(range(N_CORES)))
    LAST_RESULTS = res
    out = np.concatenate([res.results[i]["out"] for i in range(N_CORES)], axis=0)
    return np.asarray(out, dtype=np.float32)
